# revision 9
# baseline (speedup 1.0000x reference)
"""Trainium2 kernel for nn_AdaptedGNN (retrieval_knn affinity).

affinity[r, f] = (nf[2+f,2] + nf[2+f,4] + eps) / (dist(robot_r, frontier_f) + eps)

Fully data-parallel across 8 NeuronCores: core c owns frontier rows
[c*1e6, (c+1)*1e6). Host reshapes each core's slice into planar
(128, 2, 7824) f32 arrays (x|y positions and f2|f4 gains, padded by 1472
rows); the device computes squares/sqrt on ScalarE, adds/reciprocal/final
multiply on VectorE, and the +eps on GPSIMD, tiled with double buffering.
"""

import sys

for _p in ("/opt/trn_rl_repo",):
    if _p not in sys.path:
        sys.path.insert(0, _p)

import ml_dtypes
import numpy as np

import concourse.bacc as bacc
import concourse.dve_ops as dve_ops
import concourse.mybir as mybir
import concourse.tile as tile
from concourse.bass_utils import run_bass_kernel_spmd
from concourse.dve_ops import DveOp
from concourse.dve_spec import AluOp, Bin, C0, C1, C2, Spec, Src0


def _recip1_eps_ref(in0, in1, c0, c1, c2):
    xe = (in0 + c2).astype(np.float32)
    n = (~xe.view(np.int32)).view(np.float32)
    y0 = n * c0
    return (y0 * (c1 - xe * y0)).astype(np.float32)


_xe = Src0 + C2
_n = Bin(AluOp.BITWISE_NOT, _xe, _xe)
_y0 = _n * C0

# 1/(x + eps) in one DVE pass: bitwise-NOT exponent-flip seed on (x+eps) plus
# one minimax-tuned Newton step (~0.17% max rel err over [1e-6, 3]). eps rides
# the imm2 slot. 6 ALU nodes -- fits the 8-slice pipe.
RECIP1_EPS = DveOp(
    "RECIP1_EPS_ANT",
    Spec(body=_y0 * (C1 - _xe * _y0), reference=_recip1_eps_ref),
    subdim=False,
    uops_sha={"v3": "aa55afded45a0392", "v4": "7cf22af25044d172"},
)
RECIP1_C0 = -0.23549793
RECIP1_C1 = 2.00173235

if RECIP1_EPS.name not in dve_ops._SUB_OPCODE_FOR_NAME:
    dve_ops.OPS.append(RECIP1_EPS)
    dve_ops._SUB_OPCODE_FOR_NAME[RECIP1_EPS.name] = (
        dve_ops._CUSTOM_DVE_ROW_BASE + len(dve_ops.OPS) - 1
    )
    dve_ops.CUSTOM_DVE_SPECS[RECIP1_EPS.name] = RECIP1_EPS.spec

NUM_CORES = 8
EPS = 1e-6
P = 128           # SBUF partitions
WP = 7824         # per-partition elements per core (padded)
FC = 1_000_000    # real frontier rows per core
RPAD = P * WP     # 1,001,472 padded rows per core
# Tile schedule: half-size edge tiles shorten pipeline fill and drain.
TILE_SIZES = (489, 489, 1467, 1467, 1467, 1467, 489, 489)
assert sum(TILE_SIZES) == WP

_nc_cache = None


def _build():
    global _nc_cache
    if _nc_cache is not None:
        return _nc_cache

    f32 = mybir.dt.float32
    bf16 = mybir.dt.bfloat16
    Sq = mybir.ActivationFunctionType.Square
    Sqrt = mybir.ActivationFunctionType.Sqrt
    add = mybir.AluOpType.add
    mult = mybir.AluOpType.mult

    nc = bacc.Bacc(
        "TRN2", target_bir_lowering=False, debug=False, num_devices=NUM_CORES
    )
    xy_ext = nc.declare_dram_parameter("xy", [P, 2, WP], f32, isOutput=False)
    gf_ext = nc.declare_dram_parameter("gf", [P, 2, WP], bf16, isOutput=False)
    rb_ext = nc.declare_dram_parameter("rb", [P, 5], f32, isOutput=False)
    out_ext = nc.declare_dram_parameter("out", [P, 2, WP], bf16, isOutput=True)

    with tile.TileContext(nc) as tc:
        with (
            tc.tile_pool(name="const", bufs=1) as cpool,
            tc.tile_pool(name="io", bufs=3) as io,
            tc.tile_pool(name="work", bufs=3) as wk,
        ):
            nrb = cpool.tile([P, 5], f32)
            nc.sync.dma_start(nrb[:], rb_ext[:])
            # touch Square early so the ACT table load overlaps the first DMA
            warm = cpool.tile([P, 1], f32)
            nc.scalar.activation(warm[:], warm[:], Sq)
            a = 0
            for C in TILE_SIZES:
                b = a + C
                XY = io.tile([P, 2 * C], f32, tag="xy")
                nc.sync.dma_start(
                    XY[:].rearrange("p (j c) -> p j c", j=2), xy_ext[:, :, a:b]
                )
                GF = io.tile([P, 2 * C], bf16, tag="gf")
                nc.sync.dma_start(
                    GF[:].rearrange("p (j c) -> p j c", j=2), gf_ext[:, :, a:b]
                )
                X, Y = XY[:, :C], XY[:, C:]
                # (x - rx_r)^2 and (y - ry_r)^2, both robots, on ScalarE
                ABx = wk.tile([P, 2 * C], bf16, tag="abx")
                ABy = wk.tile([P, 2 * C], bf16, tag="aby")
                nc.scalar.activation(ABx[:, :C], X, Sq, bias=nrb[:, 0:1])
                nc.scalar.activation(ABx[:, C:], X, Sq, bias=nrb[:, 1:2])
                nc.scalar.activation(ABy[:, :C], Y, Sq, bias=nrb[:, 2:3])
                nc.scalar.activation(ABy[:, C:], Y, Sq, bias=nrb[:, 3:4])
                S = ABx  # in-place: dist^2 overwrites the x-squares tile
                nc.vector.tensor_tensor(S[:], ABx[:], ABy[:], add)
                D = wk.tile([P, 2 * C], f32, tag="d")
                nc.scalar.activation(D[:], S[:], Sqrt)
                R = wk.tile([P, 2 * C], f32, tag="r")
                nc.vector._custom_dve(
                    RECIP1_EPS, out=R[:], in0=D[:],
                    s0=RECIP1_C0, s1=RECIP1_C1, imm2=EPS,
                )
                G = wk.tile([P, C], bf16, tag="g")
                nc.vector.tensor_tensor(G[:], GF[:, :C], GF[:, C:], add)
                O = wk.tile([P, 2 * C], bf16, tag="o")
                nc.vector.scalar_tensor_tensor(O[:, :C], G[:], EPS, R[:, :C], add, mult)
                nc.vector.scalar_tensor_tensor(O[:, C:], G[:], EPS, R[:, C:], add, mult)
                nc.sync.dma_start(
                    out_ext[:, :, a:b], O[:].rearrange("p (j c) -> p j c", j=2)
                )
                a = b
    nc.compile()
    _nc_cache = nc
    return nc


def _plane(col: np.ndarray, pad_val: float, dtype=np.float32) -> np.ndarray:
    full = np.empty(RPAD, dtype=dtype)
    full[:FC] = col
    full[FC:] = pad_val
    return full.reshape(P, WP)


def _prepare_in_maps(node_features: np.ndarray):
    nf = np.asarray(node_features, dtype=np.float32)
    robots = nf[:2, :2]  # (2, 2): [robot, (x, y)]
    rb = np.tile(
        np.array(
            [-robots[0, 0], -robots[1, 0], -robots[0, 1], -robots[1, 1], EPS],
            dtype=np.float32,
        ),
        (P, 1),
    )
    in_maps = []
    for c in range(NUM_CORES):
        rows = nf[2 + c * FC : 2 + (c + 1) * FC]
        xy = np.stack([_plane(rows[:, 0], 2.0), _plane(rows[:, 1], 2.0)], axis=1)
        bf = ml_dtypes.bfloat16
        gf = np.stack(
            [_plane(rows[:, 2], 0.0, bf), _plane(rows[:, 4], 0.0, bf)], axis=1
        )
        in_maps.append({"xy": xy, "gf": gf, "rb": rb})
    return in_maps


def _assemble(results) -> np.ndarray:
    parts = []
    for c in range(NUM_CORES):
        o = np.asarray(results[c]["out"], dtype=np.float32)  # (P, 2, WP)
        parts.append(np.ascontiguousarray(o.transpose(1, 0, 2)).reshape(2, RPAD)[:, :FC])
    return np.concatenate(parts, axis=1)


def run(node_features, trace: bool = False):
    """Returns (affinity, BassKernelResults)."""
    nc = _build()
    in_maps = _prepare_in_maps(node_features)
    res = run_bass_kernel_spmd(nc, in_maps, list(range(NUM_CORES)), trace=trace)
    return _assemble(res.results), res


def kernel(node_features, edge_features=None, edge_indices=None):
    affinity, _ = run(node_features, trace=False)
    return affinity


# revision 10
# speedup vs baseline: 1.2484x; 1.2484x over previous
"""Trainium2 kernel for nn_AdaptedGNN (retrieval_knn affinity).

affinity[r, f] = (nf[2+f,2] + nf[2+f,4] + eps) / (dist(robot_r, frontier_f) + eps)

Fully data-parallel across 8 NeuronCores: core c owns frontier rows
[c*1e6, (c+1)*1e6). Host reshapes each core's slice into planar
(128, 2, 7824) f32 arrays (x|y positions and f2|f4 gains, padded by 1472
rows); the device computes squares/sqrt on ScalarE, adds/reciprocal/final
multiply on VectorE, and the +eps on GPSIMD, tiled with double buffering.
"""

import sys

for _p in ("/opt/trn_rl_repo",):
    if _p not in sys.path:
        sys.path.insert(0, _p)

import ml_dtypes
import numpy as np

import concourse.bacc as bacc
import concourse.dve_ops as dve_ops
import concourse.mybir as mybir
import concourse.tile as tile
from concourse.bass_utils import run_bass_kernel_spmd
from concourse.dve_ops import DveOp
from concourse.dve_spec import AluOp, Bin, C0, C1, C2, Spec, Src0


def _recip1_eps_ref(in0, in1, c0, c1, c2):
    xe = (in0 + c2).astype(np.float32)
    n = (~xe.view(np.int32)).view(np.float32)
    y0 = n * c0
    return (y0 * (c1 - xe * y0)).astype(np.float32)


_xe = Src0 + C2
_n = Bin(AluOp.BITWISE_NOT, _xe, _xe)
_y0 = _n * C0

# 1/(x + eps) in one DVE pass: bitwise-NOT exponent-flip seed on (x+eps) plus
# one minimax-tuned Newton step (~0.17% max rel err over [1e-6, 3]). eps rides
# the imm2 slot. 6 ALU nodes -- fits the 8-slice pipe.
RECIP1_EPS = DveOp(
    "RECIP1_EPS_ANT",
    Spec(body=_y0 * (C1 - _xe * _y0), reference=_recip1_eps_ref),
    subdim=False,
    uops_sha={"v3": "aa55afded45a0392", "v4": "7cf22af25044d172"},
)
RECIP1_C0 = -0.23549793
RECIP1_C1 = 2.00173235

if RECIP1_EPS.name not in dve_ops._SUB_OPCODE_FOR_NAME:
    dve_ops.OPS.append(RECIP1_EPS)
    dve_ops._SUB_OPCODE_FOR_NAME[RECIP1_EPS.name] = (
        dve_ops._CUSTOM_DVE_ROW_BASE + len(dve_ops.OPS) - 1
    )
    dve_ops.CUSTOM_DVE_SPECS[RECIP1_EPS.name] = RECIP1_EPS.spec

NUM_CORES = 8
EPS = 1e-6
P = 128           # SBUF partitions
WP = 7824         # per-partition elements per core (padded)
FC = 1_000_000    # real frontier rows per core
RPAD = P * WP     # 1,001,472 padded rows per core
# Tile schedule: half-size edge tiles shorten pipeline fill and drain.
TILE_SIZES = (489, 489, 978, 978, 978, 978, 978, 978, 489, 489)
assert sum(TILE_SIZES) == WP

_nc_cache = None


def _build():
    global _nc_cache
    if _nc_cache is not None:
        return _nc_cache

    f32 = mybir.dt.float32
    bf16 = mybir.dt.bfloat16
    Sq = mybir.ActivationFunctionType.Square
    Sqrt = mybir.ActivationFunctionType.Sqrt
    add = mybir.AluOpType.add
    mult = mybir.AluOpType.mult

    nc = bacc.Bacc(
        "TRN2", target_bir_lowering=False, debug=False, num_devices=NUM_CORES
    )
    xy_ext = nc.declare_dram_parameter("xy", [P, 2, WP], f32, isOutput=False)
    gf_ext = nc.declare_dram_parameter("gf", [P, 2, WP], bf16, isOutput=False)
    rb_ext = nc.declare_dram_parameter("rb", [P, 5], f32, isOutput=False)
    out_ext = nc.declare_dram_parameter("out", [P, 2, WP], bf16, isOutput=True)

    with tile.TileContext(nc) as tc:
        with (
            tc.tile_pool(name="const", bufs=1) as cpool,
            tc.tile_pool(name="io", bufs=4) as io,
            tc.tile_pool(name="work", bufs=4) as wk,
        ):
            nrb = cpool.tile([P, 5], f32)
            nc.sync.dma_start(nrb[:], rb_ext[:])
            # touch Square early so the ACT table load overlaps the first DMA
            warm = cpool.tile([P, 1], f32)
            nc.scalar.activation(warm[:], warm[:], Sq)
            a = 0
            for C in TILE_SIZES:
                b = a + C
                XY = io.tile([P, 2 * C], f32, tag="xy")
                nc.sync.dma_start(
                    XY[:].rearrange("p (j c) -> p j c", j=2), xy_ext[:, :, a:b]
                )
                GF = io.tile([P, 2 * C], bf16, tag="gf")
                nc.sync.dma_start(
                    GF[:].rearrange("p (j c) -> p j c", j=2), gf_ext[:, :, a:b]
                )
                X, Y = XY[:, :C], XY[:, C:]
                # (x - rx_r)^2 and (y - ry_r)^2, both robots, on ScalarE
                ABx = wk.tile([P, 2 * C], bf16, tag="abx")
                ABy = wk.tile([P, 2 * C], bf16, tag="aby")
                nc.scalar.activation(ABx[:, :C], X, Sq, bias=nrb[:, 0:1])
                nc.scalar.activation(ABx[:, C:], X, Sq, bias=nrb[:, 1:2])
                nc.scalar.activation(ABy[:, :C], Y, Sq, bias=nrb[:, 2:3])
                nc.scalar.activation(ABy[:, C:], Y, Sq, bias=nrb[:, 3:4])
                S = wk.tile([P, 2 * C], bf16, tag="s")
                nc.vector.tensor_tensor(S[:], ABx[:], ABy[:], add)
                D = wk.tile([P, 2 * C], f32, tag="d")
                nc.scalar.activation(D[:], S[:], Sqrt)
                R = wk.tile([P, 2 * C], f32, tag="r")
                nc.vector._custom_dve(
                    RECIP1_EPS, out=R[:], in0=D[:],
                    s0=RECIP1_C0, s1=RECIP1_C1, imm2=EPS,
                )
                G = wk.tile([P, C], bf16, tag="g")
                nc.vector.tensor_tensor(G[:], GF[:, :C], GF[:, C:], add)
                O = wk.tile([P, 2 * C], bf16, tag="o")
                nc.vector.scalar_tensor_tensor(O[:, :C], G[:], EPS, R[:, :C], add, mult)
                nc.vector.scalar_tensor_tensor(O[:, C:], G[:], EPS, R[:, C:], add, mult)
                nc.sync.dma_start(
                    out_ext[:, :, a:b], O[:].rearrange("p (j c) -> p j c", j=2)
                )
                a = b
    nc.compile()
    _nc_cache = nc
    return nc


def _plane(col: np.ndarray, pad_val: float, dtype=np.float32) -> np.ndarray:
    full = np.empty(RPAD, dtype=dtype)
    full[:FC] = col
    full[FC:] = pad_val
    return full.reshape(P, WP)


def _prepare_in_maps(node_features: np.ndarray):
    nf = np.asarray(node_features, dtype=np.float32)
    robots = nf[:2, :2]  # (2, 2): [robot, (x, y)]
    rb = np.tile(
        np.array(
            [-robots[0, 0], -robots[1, 0], -robots[0, 1], -robots[1, 1], EPS],
            dtype=np.float32,
        ),
        (P, 1),
    )
    in_maps = []
    for c in range(NUM_CORES):
        rows = nf[2 + c * FC : 2 + (c + 1) * FC]
        xy = np.stack([_plane(rows[:, 0], 2.0), _plane(rows[:, 1], 2.0)], axis=1)
        bf = ml_dtypes.bfloat16
        gf = np.stack(
            [_plane(rows[:, 2], 0.0, bf), _plane(rows[:, 4], 0.0, bf)], axis=1
        )
        in_maps.append({"xy": xy, "gf": gf, "rb": rb})
    return in_maps


def _assemble(results) -> np.ndarray:
    parts = []
    for c in range(NUM_CORES):
        o = np.asarray(results[c]["out"], dtype=np.float32)  # (P, 2, WP)
        parts.append(np.ascontiguousarray(o.transpose(1, 0, 2)).reshape(2, RPAD)[:, :FC])
    return np.concatenate(parts, axis=1)


def run(node_features, trace: bool = False):
    """Returns (affinity, BassKernelResults)."""
    nc = _build()
    in_maps = _prepare_in_maps(node_features)
    res = run_bass_kernel_spmd(nc, in_maps, list(range(NUM_CORES)), trace=trace)
    return _assemble(res.results), res


def kernel(node_features, edge_features=None, edge_indices=None):
    affinity, _ = run(node_features, trace=False)
    return affinity
